# revision 5
# baseline (speedup 1.0000x reference)
"""Trainium2 Bass kernel for nn_AttentionHyperNet (sparse_attention), v4.

Full-input contract: kernel(**inputs) takes FULL unsharded inputs, returns
FULL output [2048, 16, 32] f32. Shards batch across 8 cores (data parallel).

Per-core design (BS_L=256 samples, groups of S=8 samples = 512 entity tokens):
  - all GEMMs bf16 (1 cyc/row on PE); activations bf16 end-to-end
  - entities PE-transposed (bf16, into a bitcast PSUM region) -> feature-major
  - fc1/K/Q feature-major; V token-major via x1-token-slice-stationary GEMM
    (2 samples per matmul); entity mask folded into V during evacuation;
    keep-column (+1e-30) appended per head so the attention matmul emits the
    softmax denominator for free (EPS folded into the keep values)
  - logits computed TRANSPOSED per (sample, head): lhsT = kT strip, rhs = qT,
    free dim = 16 queries; softmax without max-subtraction (small logits),
    no masking on the exp (mask lives in V/keep)
  - attn matmul: lhsT = exp'd logitsT (widened to 32 cols so full 32-row
    strips are written), rhs = Vhat [64k, 33] -> token-major attn +
    denominator; normalize via DVE reciprocal + free-axis broadcast;
    PE-transpose (bf16 psum) back to feature-major for Wout/fc2
  - agent post-mask applied once at the end (subsumes intermediate masks)

HW constraint (found empirically, crashes the device otherwise): concurrent
tile_position-packed matmuls on DIFFERENT row strips may share a column
strip only if they write DIFFERENT PSUM banks. Hence the logits PSUM is a
4-bank tile with bank = head%4 (the contraction row strip), and the exp
gathers the sparse bank layout back into a dense ET tile. The spare columns
of those banks host the entity-transpose and Q PSUM. Attention matmuls are
naturally safe (same-column pairs always share the row strip or the bank
differs). Work is spread across ACT/DVE (Pool: SBUF-only copies/memsets).
"""

import math
import numpy as np
from contextlib import ExitStack

import concourse.bass as bass
import concourse.mybir as mybir
import concourse.tile as tile
from concourse import bacc
from concourse.masks import make_identity

F32 = mybir.dt.float32
BF16 = mybir.dt.bfloat16
I32 = mybir.dt.int32
AF = mybir.ActivationFunctionType
ALU = mybir.AluOpType

BS, NE, NA, ED, H, NH, M = 2048, 64, 16, 128, 256, 8, 32
HD = H // NH  # 32
N_CORES = 8
BS_L = BS // N_CORES  # 256
S = 8                 # samples per group
NG = BS_L // S        # 32 groups
SCALE = 1.0 / math.sqrt(HD)
EPS = 1e-30


def build_nc(bs_l=BS_L):
    ng = bs_l // S
    nc = bacc.Bacc("TRN2", target_bir_lowering=False)

    ent_d = nc.dram_tensor("entities", [bs_l, NE, ED], F32, kind="ExternalInput").ap()
    em_d = nc.dram_tensor("entity_mask", [bs_l, NE], I32, kind="ExternalInput").ap()
    w1_d = nc.dram_tensor("W1", [H, ED], F32, kind="ExternalInput").ap()
    b1_d = nc.dram_tensor("b1", [H], F32, kind="ExternalInput").ap()
    win_d = nc.dram_tensor("Win", [3 * H, H], F32, kind="ExternalInput").ap()
    wout_d = nc.dram_tensor("Wout", [H, H], F32, kind="ExternalInput").ap()
    bout_d = nc.dram_tensor("bout", [H], F32, kind="ExternalInput").ap()
    w2_d = nc.dram_tensor("W2", [M, H], F32, kind="ExternalInput").ap()
    b2_d = nc.dram_tensor("b2", [M], F32, kind="ExternalInput").ap()
    out_d = nc.dram_tensor("out", [bs_l, NA, M], F32, kind="ExternalOutput").ap()

    with tile.TileContext(nc) as tc, ExitStack() as ctx:
        # ---------------- pools ----------------
        wpool = ctx.enter_context(tc.tile_pool(name="weights", bufs=1))
        pre = ctx.enter_context(tc.tile_pool(name="pre", bufs=2))
        entp = ctx.enter_context(tc.tile_pool(name="entp", bufs=5))
        eTp = ctx.enter_context(tc.tile_pool(name="eTp", bufs=3))
        x1p = ctx.enter_context(tc.tile_pool(name="x1p", bufs=3))
        ktp = ctx.enter_context(tc.tile_pool(name="ktp", bufs=3))
        qtp = ctx.enter_context(tc.tile_pool(name="qtp", bufs=3))
        vbp = ctx.enter_context(tc.tile_pool(name="vbp", bufs=3))
        etp = ctx.enter_context(tc.tile_pool(name="etp", bufs=3))
        smp = ctx.enter_context(tc.tile_pool(name="smp", bufs=3))
        anp = ctx.enter_context(tc.tile_pool(name="anp", bufs=3))
        atp = ctx.enter_context(tc.tile_pool(name="atp", bufs=3))
        aop = ctx.enter_context(tc.tile_pool(name="aop", bufs=3))
        otp = ctx.enter_context(tc.tile_pool(name="otp", bufs=3))

        psS = ctx.enter_context(tc.tile_pool(name="psS", bufs=2, space="PSUM"))
        psL = ctx.enter_context(tc.tile_pool(name="psL", bufs=1, space="PSUM"))
        psA = ctx.enter_context(tc.tile_pool(name="psA", bufs=2, space="PSUM"))

        # ---------------- preamble ----------------
        ent_tiles = {}

        def emit_ent_dma(g):
            ent = entp.tile([128, 512], F32, tag="ent", name=f"ent{g}")
            nc.sync.dma_start(
                out=ent[:].rearrange("p (tb e) -> p tb e", tb=4),
                in_=ent_d[g * S : (g + 1) * S]
                .rearrange("s n e -> (s n) e")
                .rearrange("(tb p) e -> p tb e", p=128),
            )
            ent_tiles[g] = ent

        for _g in range(3):
            emit_ent_dma(_g)

        identF = wpool.tile([128, 128], F32, tag="identF")
        make_identity(nc, identF[:])
        identB = wpool.tile([128, 128], BF16, tag="identB")
        make_identity(nc, identB[:])

        # raw weight loads (fp32, row-tiled into one sbuf tile each)
        w1raw = pre.tile([128, 256], F32, tag="w1raw", name="w1raw")
        nc.sync.dma_start(
            out=w1raw[:].rearrange("p (t e) -> p t e", t=2),
            in_=w1_d.rearrange("(t p) e -> p t e", p=128),
        )
        winraw = pre.tile([128, 1536], F32, tag="winraw", name="winraw")
        for w3 in range(3):
            nc.sync.dma_start(
                out=winraw[:, w3 * 512 : (w3 + 1) * 512]
                .rearrange("p (t h) -> p t h", t=2),
                in_=win_d.rearrange("(t p) h -> p t h", p=128)[:, 2 * w3 : 2 * w3 + 2],
            )
        woraw = pre.tile([128, 512], F32, tag="woraw", name="woraw")
        nc.sync.dma_start(
            out=woraw[:].rearrange("p (t h) -> p t h", t=2),
            in_=wout_d.rearrange("(t p) h -> p t h", p=128),
        )
        w2raw = pre.tile([32, 256], F32, tag="w2raw", name="w2raw")
        nc.sync.dma_start(out=w2raw[:], in_=w2_d)

        # W1T [ED, H] bf16
        w1T = wpool.tile([128, 256], BF16, tag="w1T")
        ps = psS.tile([128, 512], F32, tag="ps", name="psw1")
        for t in range(2):
            nc.tensor.transpose(
                ps[:, t * 128 : (t + 1) * 128], w1raw[:, t * 128 : (t + 1) * 128],
                identF[:],
            )
        nc.scalar.activation(w1T[:], ps[:, :256], AF.Copy)

        def make_wT(raw_ap, t0, tag):
            """rows [t0*128:(t0+2)*128] of a [*,256] weight -> transposed bf16
            tiles XT[ib] [128 (in-half), 256 (out)]."""
            tiles = []
            for ib in range(2):
                t = wpool.tile([128, 256], BF16, tag=f"{tag}{ib}", name=f"{tag}{ib}")
                ps = psS.tile([128, 512], F32, tag="ps", name=f"psw{tag}{ib}")
                for ta in range(2):
                    nc.tensor.transpose(
                        ps[:, ta * 128 : (ta + 1) * 128],
                        raw_ap[:, (t0 + ta) * 256 + ib * 128 : (t0 + ta) * 256 + (ib + 1) * 128],
                        identF[:],
                    )
                nc.scalar.activation(t[:], ps[:, :256], AF.Copy)
                tiles.append(t)
            return tiles

        wqT = make_wT(winraw, 0, "wqT")
        wkT = make_wT(winraw, 2, "wkT")
        wvT = make_wT(winraw, 4, "wvT")
        woT = make_wT(woraw, 0, "woT")

        w2T = []
        for ib in range(2):
            t = wpool.tile([128, 32], BF16, tag=f"w2T{ib}", name=f"w2T{ib}")
            ps = psS.tile([128, 512], F32, tag="ps", name=f"psw2{ib}")
            nc.tensor.transpose(
                ps[:, :32], w2raw[:, ib * 128 : (ib + 1) * 128], identF[:32, :32]
            )
            nc.vector.tensor_copy(t[:], ps[:, :32])
            w2T.append(t)

        b1sb = wpool.tile([128, 2], F32, tag="b1sb")
        nc.sync.dma_start(out=b1sb[:], in_=b1_d.rearrange("(t p) -> p t", p=128))
        bosb = wpool.tile([128, 2], F32, tag="bosb")
        nc.sync.dma_start(out=bosb[:], in_=bout_d.rearrange("(t p) -> p t", p=128))
        b2sb = wpool.tile([32, 1], F32, tag="b2sb")
        nc.sync.dma_start(out=b2sb[:], in_=b2_d.rearrange("(p o) -> p o", o=1))

        # keep masks: keepV [128=(sp,k), bs_l//2 (sample-pair)] = 1 - em
        emV = pre.tile([128, bs_l // 2], I32, tag="emV", name="emV")
        nc.sync.dma_start(
            out=emV[:], in_=em_d.rearrange("(p two) n -> (two n) p", two=2)
        )
        keepVf = wpool.tile([128, bs_l // 2], F32, tag="keepVf")
        nc.vector.tensor_scalar(
            out=keepVf[:], in0=emV[:], scalar1=-1.0, scalar2=1.0,
            op0=ALU.mult, op1=ALU.add,
        )
        keepVb = wpool.tile([128, bs_l // 2], BF16, tag="keepVb")
        nc.gpsimd.tensor_scalar_add(keepVb[:], keepVf[:], 1e-30)

        # keepA [128=(s,q), ng] for the final agent post-mask
        emA = pre.tile([128, ng], I32, tag="emA", name="emA")
        for s8 in range(S):
            nc.sync.dma_start(
                out=emA[s8 * NA : (s8 + 1) * NA, :],
                in_=em_d[:, 0:NA].rearrange("(g s) q -> s q g", s=S)[s8],
            )
        keepAf = wpool.tile([128, ng], F32, tag="keepAf")
        nc.vector.tensor_scalar(
            out=keepAf[:], in0=emA[:], scalar1=-1.0, scalar2=1.0,
            op0=ALU.mult, op1=ALU.add,
        )

        # ---------------- main loop ----------------

        st = {}  # per-group state

        def front_a(g):
            ent = ent_tiles.pop(g)
            entb = entp.tile([128, 512], BF16, tag="entb", name=f"entb{g}")
            nc.gpsimd.tensor_copy(entb[:], ent[:])
            psLg = psL.tile([128, 2048], F32, tag="l", name=f"psL{g}")
            psEb = psLg[:].bitcast(BF16)[:, 256:768]
            for tb in range(4):
                nc.tensor.transpose(
                    psEb[:, tb * 128 : (tb + 1) * 128],
                    entb[:, tb * 128 : (tb + 1) * 128],
                    identB[:],
                )
            eT = eTp.tile([128, 512], BF16, tag="eT", name=f"eT{g}")
            nc.vector.tensor_copy(eT[:], psEb[:])
            st[g] = dict(eT=eT, psLg=psLg)

        def front_b(g):
            s_ = st[g]
            x1 = []
            for hb in range(2):
                psX = psS.tile([128, 512], F32, tag="ps", name=f"psX{g}_{hb}")
                nc.tensor.matmul(
                    psX[:], w1T[:, hb * 128 : (hb + 1) * 128], s_["eT"][:],
                    start=True, stop=True,
                )
                t = x1p.tile([128, 512], BF16, tag=f"x1{hb}", name=f"x1_{g}_{hb}")
                nc.scalar.activation(t[:], psX[:], AF.Relu, bias=b1sb[:, hb : hb + 1])
                x1.append(t)
            s_["x1"] = x1

        def front_c(g):
            s_ = st[g]
            x1 = s_["x1"]
            kT = []
            for hb in range(2):
                psK = psS.tile([128, 512], F32, tag="ps", name=f"psK{g}_{hb}")
                for ib in range(2):
                    nc.tensor.matmul(
                        psK[:], wkT[ib][:, hb * 128 : (hb + 1) * 128], x1[ib][:],
                        start=(ib == 0), stop=(ib == 1),
                    )
                t = ktp.tile([128, 512], BF16, tag=f"kT{hb}", name=f"kT{g}_{hb}")
                if hb == 0:
                    nc.scalar.activation(t[:], psK[:], AF.Copy)
                else:
                    nc.vector.tensor_copy(t[:], psK[:])
                kT.append(t)
            psQ = s_["psLg"][:, 640:896]
            for hb in range(2):
                for ib in range(2):
                    nc.tensor.matmul(
                        psQ[:, hb * 128 : (hb + 1) * 128],
                        wqT[ib][:, hb * 128 : (hb + 1) * 128],
                        x1[ib][:].rearrange("p (s n) -> p s n", s=S)[:, :, :NA],
                        start=(ib == 0), stop=(ib == 1),
                    )
            qT = qtp.tile([128, 256], BF16, tag="qT", name=f"qT{g}")
            nc.scalar.activation(qT[:], psQ[:, :256], AF.Copy)
            vb = []
            for half in range(2):
                psV = psS.tile([128, 512], F32, tag="ps", name=f"psV{g}_{half}")
                for p2 in range(2):
                    p = half * 2 + p2
                    for ib in range(2):
                        nc.tensor.matmul(
                            psV[:, p2 * 256 : (p2 + 1) * 256],
                            x1[ib][:, p * 128 : (p + 1) * 128],
                            wvT[ib][:],
                            start=(ib == 0), stop=(ib == 1),
                        )
                t = vbp.tile([128, 2 * 264], BF16, tag=f"vb{half}", name=f"vb{g}_{half}")
                pg0 = g * 4 + half * 2
                nc.vector.tensor_tensor(
                    out=t[:].rearrange("p (c h e) -> p c h e", c=2, e=33)[:, :, :, :32],
                    in0=psV[:].rearrange("p (c h d) -> p c h d", c=2, h=NH),
                    in1=keepVf[:, pg0 : pg0 + 2]
                    .unsqueeze(2).unsqueeze(3).broadcast_to([128, 2, NH, HD]),
                    op=ALU.mult,
                )
                nc.gpsimd.tensor_copy(
                    t[:].rearrange("p (c h e) -> p c h e", c=2, e=33)[:, :, :, 32],
                    keepVb[:, pg0 : pg0 + 2].unsqueeze(2).broadcast_to([128, 2, NH]),
                )
                vb.append(t)
            s_["kT"], s_["qT"], s_["vb"] = kT, qT, vb

        def mid_d(g):
            s_ = st[g]
            kT, qT = s_["kT"], s_["qT"]
            psLg = s_["psLg"]
            for s in range(S):
                c = s % 2
                for h in range(NH):
                    hb, j = h // 4, h % 4
                    col = 512 * j + (s // 2) * 32 + hb * 16
                    nc.tensor.matmul(
                        psLg[64 * c : 64 * c + 64, col : col + 16],
                        kT[hb][32 * j : 32 * j + 32, s * 64 : (s + 1) * 64],
                        qT[32 * j : 32 * j + 32, hb * 128 + s * 16 : hb * 128 + s * 16 + 16],
                        start=True, stop=True,
                        tile_position=(32 * j, 64 * c),
                    )
            ET = etp.tile([128, 544], BF16, tag="ET", name=f"ET{g}")
            psl_v = (
                psLg[:].rearrange("p (j r) -> p j r", j=4)[:, :, 0:128]
                .rearrange("p j (p2 hb q) -> p p2 hb j q", p2=4, hb=2)
            )
            et_v = ET[:, :512].rearrange(
                "p (p2 hb j q) -> p p2 hb j q", p2=4, hb=2, j=4
            )
            nc.scalar.activation(et_v[:, :2], psl_v[:, :2], AF.Exp, scale=SCALE)
            nc.scalar.activation(et_v[:, 2:], psl_v[:, 2:], AF.Exp, scale=SCALE)
            nc.gpsimd.memset(ET[:, 512:544], 0.0)
            s_["ET"] = ET

        def mid_e(g):
            s_ = st[g]
            ET, vb = s_["ET"], s_["vb"]
            psAt = [
                psA.tile([128, 512], F32, tag="a", name=f"psA{g}_{b}")
                for b in range(2)
            ]
            for s in range(S):
                b, strip, c = s // 4, s % 4, s % 2
                half, p2 = s // 4, (s // 2) % 2
                for h in range(NH):
                    slot = (s // 2) * 8 + h
                    nc.tensor.matmul(
                        psAt[b][32 * strip : 32 * strip + 32, 33 * h : 33 * h + 33],
                        ET[64 * c : 64 * c + 64, slot * 16 : slot * 16 + 32],
                        vb[half][64 * c : 64 * c + 64, p2 * 264 + 33 * h : p2 * 264 + 33 * h + 33],
                        start=True, stop=True,
                        tile_position=(64 * c, 32 * strip),
                    )
            s_["psAt"] = psAt

        def mid_f1(g):
            s_ = st[g]
            psAt = s_["psAt"]
            an = []
            for b in range(2):
                rec = smp.tile([128, 8], F32, tag=f"rec{b}", name=f"rec{g}_{b}")
                nc.vector.reciprocal(
                    rec[:],
                    psAt[b][:, :264].rearrange("p (h e) -> p h e", e=33)[:, :, 32],
                )
                t = anp.tile([128, 256], BF16, tag=f"an{b}", name=f"an{g}_{b}")
                nc.vector.tensor_tensor(
                    out=t[:].rearrange("p (h d) -> p h d", h=NH),
                    in0=psAt[b][:, :264].rearrange("p (h e) -> p h e", e=33)[:, :, :32],
                    in1=rec[:].unsqueeze(2).broadcast_to([128, NH, HD]),
                    op=ALU.mult,
                )
                an.append(t)
            s_["an"] = an

        def mid_f2(g):
            s_ = st[g]
            an = s_["an"]
            psT = psA.tile([128, 512], F32, tag="a", name=f"psT{g}")
            psTb = psT[:].bitcast(BF16)
            for hb in range(2):
                for b in range(2):
                    nc.tensor.transpose(
                        psTb[:, hb * 256 + b * 128 : hb * 256 + (b + 1) * 128],
                        an[b][:, hb * 128 : (hb + 1) * 128],
                        identB[:],
                    )
            aT = []
            for hb in range(2):
                t = atp.tile([128, 256], BF16, tag=f"aT{hb}", name=f"aT{g}_{hb}")
                nc.vector.tensor_copy(t[:], psTb[:, hb * 256 : (hb + 1) * 256])
                aT.append(t)
            s_["aT"] = aT

        def back_g1(g):
            s_ = st[g]
            aT = s_["aT"]
            psO = psA.tile([128, 512], F32, tag="a", name=f"psO{g}")
            for ob in range(2):
                for hb in range(2):
                    nc.tensor.matmul(
                        psO[:, ob * 128 : (ob + 1) * 128],
                        woT[hb][:, ob * 128 : (ob + 1) * 128],
                        aT[hb][:]
                        .rearrange("p (b st q) -> p b st q", b=2, q=32)[:, :, :, :16],
                        start=(hb == 0), stop=(hb == 1),
                    )
            ao = []
            for ob in range(2):
                t = aop.tile([128, 128], BF16, tag=f"ao{ob}", name=f"ao{g}_{ob}")
                nc.scalar.activation(
                    t[:], psO[:, ob * 128 : (ob + 1) * 128], AF.Identity,
                    bias=bosb[:, ob : ob + 1],
                )
                ao.append(t)
            s_["psO"], s_["ao"] = psO, ao

        def back_g2(g):
            s_ = st.pop(g)
            psO, ao = s_["psO"], s_["ao"]
            for ib in range(2):
                nc.tensor.matmul(
                    psO[0:M, 256:384], w2T[ib][:], ao[ib][:],
                    start=(ib == 0), stop=(ib == 1),
                )
            ot = otp.tile([M, 128], BF16, tag="ot", name=f"ot{g}")
            nc.scalar.activation(ot[:], psO[0:M, 256:384], AF.Identity, bias=b2sb[:])
            psFb = psO[:].bitcast(BF16)
            nc.tensor.transpose(psFb[:, 768:800], ot[:], identB[:M, :M])
            osb = otp.tile([128, M], F32, tag="osb", name=f"osb{g}")
            nc.scalar.activation(
                osb[:], psFb[:, 768:800], AF.Copy, scale=keepAf[:, g : g + 1]
            )
            nc.sync.dma_start(
                out=out_d.rearrange("b q m -> (b q) m")[g * 128 : (g + 1) * 128],
                in_=osb[:],
            )

        for i in range(ng + 2):
            if i < ng:
                if i + 3 < ng:
                    emit_ent_dma(i + 3)
                front_a(i)
                front_b(i)
            if 0 <= i - 1 < ng:
                mid_e(i - 1)
                mid_f1(i - 1)
            if i < ng:
                front_c(i)
            if 0 <= i - 1 < ng:
                mid_f2(i - 1)
            if i < ng:
                mid_d(i)
            if 0 <= i - 1 < ng:
                back_g1(i - 1)
            if 0 <= i - 2 < ng:
                back_g2(i - 2)

    nc.compile()
    return nc


_NC_CACHE = {}


def get_nc(bs_l=BS_L):
    if bs_l not in _NC_CACHE:
        _NC_CACHE[bs_l] = build_nc(bs_l)
    return _NC_CACHE[bs_l]


def kernel(entities, entity_mask, W1, b1, Win, Wout, bout, W2, b2):
    from concourse.bass_utils import run_bass_kernel_spmd

    entities = np.ascontiguousarray(np.asarray(entities), dtype=np.float32)
    entity_mask = np.ascontiguousarray(np.asarray(entity_mask), dtype=np.int32)
    weights = dict(
        W1=np.asarray(W1, np.float32), b1=np.asarray(b1, np.float32),
        Win=np.asarray(Win, np.float32), Wout=np.asarray(Wout, np.float32),
        bout=np.asarray(bout, np.float32), W2=np.asarray(W2, np.float32),
        b2=np.asarray(b2, np.float32),
    )
    nc = get_nc()
    in_maps = []
    for c in range(N_CORES):
        sl = slice(c * BS_L, (c + 1) * BS_L)
        in_maps.append(
            dict(entities=entities[sl], entity_mask=entity_mask[sl], **weights)
        )
    res = run_bass_kernel_spmd(nc, in_maps, core_ids=list(range(N_CORES)))
    outs = [res.results[c]["out"].reshape(BS_L, NA, M) for c in range(N_CORES)]
    return np.concatenate(outs, axis=0)


# revision 6
# speedup vs baseline: 1.0280x; 1.0280x over previous
"""Trainium2 Bass kernel for nn_AttentionHyperNet (sparse_attention).

Full-input contract: kernel(**inputs) takes FULL unsharded inputs, returns
FULL output [2048, 16, 32] f32. Shards batch across 8 cores (data parallel).

Per-core design (BS_L=256 samples, groups of S=8 samples = 512 entity tokens):
  - all GEMMs in bf16 (1 cyc/row on PE), activations bf16 end-to-end
  - entities PE-transposed to feature-major eT; fc1/K/Q feature-major
  - V token-major via x1-token-slice-stationary GEMM (2 samples per matmul);
    entity mask folded into V during PSUM evacuation (free multiply);
    a per-head "keep" column appended to V (denominator comes out of the
    attention matmul for free)
  - logits computed TRANSPOSED (lhsT = kT strips, rhs = qT, free dim = 16
    queries) -> softmax without max-subtraction (logits are small), no
    masking needed on the exp (mask lives in V/keep)
  - attn matmul: lhsT = exp'd logitsT (stationary), rhs = Vhat [64k, 33]
    -> token-major attn + denominator; normalize via reciprocal broadcast
    on the free axis; PE-transpose back to feature-major for Wout
  - agent post-mask applied once at the end (subsumes intermediate masks)
Work is spread across ACT/DVE engines (Pool takes small SBUF-only copies).
"""

import math
import numpy as np
from contextlib import ExitStack

import concourse.bass as bass
import concourse.mybir as mybir
import concourse.tile as tile
from concourse import bacc
from concourse.masks import make_identity

F32 = mybir.dt.float32
BF16 = mybir.dt.bfloat16
I32 = mybir.dt.int32
AF = mybir.ActivationFunctionType
ALU = mybir.AluOpType

BS, NE, NA, ED, H, NH, M = 2048, 64, 16, 128, 256, 8, 32
HD = H // NH  # 32
N_CORES = 8
BS_L = BS // N_CORES  # 256
S = 8                 # samples per group
NG = BS_L // S        # 32 groups
SCALE = 1.0 / math.sqrt(HD)
EPS = 1e-30


def build_nc(bs_l=BS_L):
    ng = bs_l // S
    nc = bacc.Bacc("TRN2", target_bir_lowering=False)

    ent_d = nc.dram_tensor("entities", [bs_l, NE, ED], F32, kind="ExternalInput").ap()
    em_d = nc.dram_tensor("entity_mask", [bs_l, NE], I32, kind="ExternalInput").ap()
    w1_d = nc.dram_tensor("W1", [H, ED], F32, kind="ExternalInput").ap()
    b1_d = nc.dram_tensor("b1", [H], F32, kind="ExternalInput").ap()
    win_d = nc.dram_tensor("Win", [3 * H, H], F32, kind="ExternalInput").ap()
    wout_d = nc.dram_tensor("Wout", [H, H], F32, kind="ExternalInput").ap()
    bout_d = nc.dram_tensor("bout", [H], F32, kind="ExternalInput").ap()
    w2_d = nc.dram_tensor("W2", [M, H], F32, kind="ExternalInput").ap()
    b2_d = nc.dram_tensor("b2", [M], F32, kind="ExternalInput").ap()
    out_d = nc.dram_tensor("out", [bs_l, NA, M], F32, kind="ExternalOutput").ap()

    with tile.TileContext(nc) as tc, ExitStack() as ctx:
        # ---------------- pools ----------------
        wpool = ctx.enter_context(tc.tile_pool(name="weights", bufs=1))
        pre = ctx.enter_context(tc.tile_pool(name="pre", bufs=2))
        entp = ctx.enter_context(tc.tile_pool(name="entp", bufs=5))
        eTp = ctx.enter_context(tc.tile_pool(name="eTp", bufs=3))
        x1p = ctx.enter_context(tc.tile_pool(name="x1p", bufs=3))
        ktp = ctx.enter_context(tc.tile_pool(name="ktp", bufs=3))
        qtp = ctx.enter_context(tc.tile_pool(name="qtp", bufs=3))
        vbp = ctx.enter_context(tc.tile_pool(name="vbp", bufs=3))
        etp = ctx.enter_context(tc.tile_pool(name="etp", bufs=3))
        smp = ctx.enter_context(tc.tile_pool(name="smp", bufs=3))
        anp = ctx.enter_context(tc.tile_pool(name="anp", bufs=3))
        atp = ctx.enter_context(tc.tile_pool(name="atp", bufs=3))
        aop = ctx.enter_context(tc.tile_pool(name="aop", bufs=3))
        otp = ctx.enter_context(tc.tile_pool(name="otp", bufs=3))

        psS = ctx.enter_context(tc.tile_pool(name="psS", bufs=2, space="PSUM"))
        psL = ctx.enter_context(tc.tile_pool(name="psL", bufs=1, space="PSUM"))
        psA = ctx.enter_context(tc.tile_pool(name="psA", bufs=2, space="PSUM"))

        # ---------------- preamble ----------------
        ent_tiles = {}

        def emit_ent_dma(g):
            ent = entp.tile([128, 512], F32, tag="ent", name=f"ent{g}")
            nc.sync.dma_start(
                out=ent[:].rearrange("p (tb e) -> p tb e", tb=4),
                in_=ent_d[g * S : (g + 1) * S]
                .rearrange("s n e -> (s n) e")
                .rearrange("(tb p) e -> p tb e", p=128),
            )
            ent_tiles[g] = ent

        for _g in range(3):
            emit_ent_dma(_g)

        identF = wpool.tile([128, 128], F32, tag="identF")
        make_identity(nc, identF[:])
        identB = wpool.tile([128, 128], BF16, tag="identB")
        make_identity(nc, identB[:])

        # raw weight loads (fp32, row-tiled into one sbuf tile each)
        w1raw = pre.tile([128, 256], F32, tag="w1raw", name="w1raw")
        nc.sync.dma_start(
            out=w1raw[:].rearrange("p (t e) -> p t e", t=2),
            in_=w1_d.rearrange("(t p) e -> p t e", p=128),
        )
        winraw = pre.tile([128, 1536], F32, tag="winraw", name="winraw")
        for w3 in range(3):
            nc.sync.dma_start(
                out=winraw[:, w3 * 512 : (w3 + 1) * 512]
                .rearrange("p (t h) -> p t h", t=2),
                in_=win_d.rearrange("(t p) h -> p t h", p=128)[:, 2 * w3 : 2 * w3 + 2],
            )
        woraw = pre.tile([128, 512], F32, tag="woraw", name="woraw")
        nc.sync.dma_start(
            out=woraw[:].rearrange("p (t h) -> p t h", t=2),
            in_=wout_d.rearrange("(t p) h -> p t h", p=128),
        )
        w2raw = pre.tile([32, 256], F32, tag="w2raw", name="w2raw")
        nc.sync.dma_start(out=w2raw[:], in_=w2_d)

        # W1T [ED, H] bf16
        w1T = wpool.tile([128, 256], BF16, tag="w1T")
        ps = psS.tile([128, 512], F32, tag="ps", name="psw1")
        for t in range(2):
            nc.tensor.transpose(
                ps[:, t * 128 : (t + 1) * 128], w1raw[:, t * 128 : (t + 1) * 128],
                identF[:],
            )
        nc.scalar.activation(w1T[:], ps[:, :256], AF.Copy)

        def make_wT(raw_ap, t0, tag):
            """rows [t0*128:(t0+2)*128] of a [*,256] weight -> transposed bf16
            tiles XT[ib] [128 (in-half), 256 (out)]."""
            tiles = []
            for ib in range(2):
                t = wpool.tile([128, 256], BF16, tag=f"{tag}{ib}", name=f"{tag}{ib}")
                ps = psS.tile([128, 512], F32, tag="ps", name=f"psw{tag}{ib}")
                for ta in range(2):
                    nc.tensor.transpose(
                        ps[:, ta * 128 : (ta + 1) * 128],
                        raw_ap[:, (t0 + ta) * 256 + ib * 128 : (t0 + ta) * 256 + (ib + 1) * 128],
                        identF[:],
                    )
                nc.scalar.activation(t[:], ps[:, :256], AF.Copy)
                tiles.append(t)
            return tiles

        wqT = make_wT(winraw, 0, "wqT")
        wkT = make_wT(winraw, 2, "wkT")
        wvT = make_wT(winraw, 4, "wvT")
        woT = make_wT(woraw, 0, "woT")

        w2T = []
        for ib in range(2):
            t = wpool.tile([128, 32], BF16, tag=f"w2T{ib}", name=f"w2T{ib}")
            ps = psS.tile([128, 512], F32, tag="ps", name=f"psw2{ib}")
            nc.tensor.transpose(
                ps[:, :32], w2raw[:, ib * 128 : (ib + 1) * 128], identF[:32, :32]
            )
            nc.vector.tensor_copy(t[:], ps[:, :32])
            w2T.append(t)

        b1sb = wpool.tile([128, 2], F32, tag="b1sb")
        nc.sync.dma_start(out=b1sb[:], in_=b1_d.rearrange("(t p) -> p t", p=128))
        bosb = wpool.tile([128, 2], F32, tag="bosb")
        nc.sync.dma_start(out=bosb[:], in_=bout_d.rearrange("(t p) -> p t", p=128))
        b2sb = wpool.tile([32, 1], F32, tag="b2sb")
        nc.sync.dma_start(out=b2sb[:], in_=b2_d.rearrange("(p o) -> p o", o=1))

        # keep masks: keepV [128=(sp,k), bs_l//2 (sample-pair)] = 1 - em
        emV = pre.tile([128, bs_l // 2], I32, tag="emV", name="emV")
        nc.sync.dma_start(
            out=emV[:], in_=em_d.rearrange("(p two) n -> (two n) p", two=2)
        )
        keepVf = wpool.tile([128, bs_l // 2], F32, tag="keepVf")
        nc.vector.tensor_scalar(
            out=keepVf[:], in0=emV[:], scalar1=-1.0, scalar2=1.0,
            op0=ALU.mult, op1=ALU.add,
        )
        keepVb = wpool.tile([128, bs_l // 2], BF16, tag="keepVb")
        nc.gpsimd.tensor_scalar_add(keepVb[:], keepVf[:], 1e-30)

        # keepA [128=(s,q), ng] for the final agent post-mask
        emA = pre.tile([128, ng], I32, tag="emA", name="emA")
        for s8 in range(S):
            nc.sync.dma_start(
                out=emA[s8 * NA : (s8 + 1) * NA, :],
                in_=em_d[:, 0:NA].rearrange("(g s) q -> s q g", s=S)[s8],
            )
        keepAf = wpool.tile([128, ng], F32, tag="keepAf")
        nc.vector.tensor_scalar(
            out=keepAf[:], in0=emA[:], scalar1=-1.0, scalar2=1.0,
            op0=ALU.mult, op1=ALU.add,
        )

        # ---------------- main loop ----------------

        st = {}  # per-group state

        def front_a(g):
            ent = ent_tiles.pop(g)
            entb = entp.tile([128, 512], BF16, tag="entb", name=f"entb{g}")
            nc.gpsimd.tensor_copy(entb[:], ent[:])
            psLg = psL.tile([128, 2048], F32, tag="l", name=f"psL{g}")
            psEb = psLg[:].bitcast(BF16)[:, 256:768]
            for tb in range(4):
                nc.tensor.transpose(
                    psEb[:, tb * 128 : (tb + 1) * 128],
                    entb[:, tb * 128 : (tb + 1) * 128],
                    identB[:],
                )
            eT = eTp.tile([128, 512], BF16, tag="eT", name=f"eT{g}")
            nc.vector.tensor_copy(eT[:], psEb[:])
            st[g] = dict(eT=eT, psLg=psLg)

        def front_b(g):
            s_ = st[g]
            x1 = []
            for hb in range(2):
                psX = psS.tile([128, 512], F32, tag="ps", name=f"psX{g}_{hb}")
                nc.tensor.matmul(
                    psX[:], w1T[:, hb * 128 : (hb + 1) * 128], s_["eT"][:],
                    start=True, stop=True,
                )
                t = x1p.tile([128, 512], BF16, tag=f"x1{hb}", name=f"x1_{g}_{hb}")
                nc.scalar.activation(t[:], psX[:], AF.Relu, bias=b1sb[:, hb : hb + 1])
                x1.append(t)
            s_["x1"] = x1

        def front_c(g):
            s_ = st[g]
            x1 = s_["x1"]
            kTm = ktp.tile([128, 1024], BF16, tag="kTm", name=f"kT{g}")
            psKa = psS.tile([128, 512], F32, tag="ps", name=f"psK{g}_0")
            psKb = psS.tile([128, 512], F32, tag="ps", name=f"psK{g}_1")
            for hb, psK in enumerate((psKa, psKb)):
                for ib in range(2):
                    nc.tensor.matmul(
                        psK[:], wkT[ib][:, hb * 128 : (hb + 1) * 128], x1[ib][:],
                        start=(ib == 0), stop=(ib == 1),
                    )
                if hb == 0:
                    nc.scalar.activation(kTm[:, :512], psK[:], AF.Copy)
                else:
                    nc.vector.tensor_copy(kTm[:, 512:], psK[:])
            kT = [kTm[:, :512], kTm[:, 512:]]
            psQ = s_["psLg"][:, 640:896]
            for hb in range(2):
                for ib in range(2):
                    nc.tensor.matmul(
                        psQ[:, hb * 128 : (hb + 1) * 128],
                        wqT[ib][:, hb * 128 : (hb + 1) * 128],
                        x1[ib][:].rearrange("p (s n) -> p s n", s=S)[:, :, :NA],
                        start=(ib == 0), stop=(ib == 1),
                    )
            qT = qtp.tile([128, 256], BF16, tag="qT", name=f"qT{g}")
            nc.scalar.activation(qT[:], psQ[:, :256], AF.Copy)
            vb = []
            for half in range(2):
                psV = psS.tile([128, 512], F32, tag="ps", name=f"psV{g}_{half}")
                for p2 in range(2):
                    p = half * 2 + p2
                    for ib in range(2):
                        nc.tensor.matmul(
                            psV[:, p2 * 256 : (p2 + 1) * 256],
                            x1[ib][:, p * 128 : (p + 1) * 128],
                            wvT[ib][:],
                            start=(ib == 0), stop=(ib == 1),
                        )
                t = vbp.tile([128, 2 * 264], BF16, tag=f"vb{half}", name=f"vb{g}_{half}")
                pg0 = g * 4 + half * 2
                nc.vector.tensor_tensor(
                    out=t[:].rearrange("p (c h e) -> p c h e", c=2, e=33)[:, :, :, :32],
                    in0=psV[:].rearrange("p (c h d) -> p c h d", c=2, h=NH),
                    in1=keepVf[:, pg0 : pg0 + 2]
                    .unsqueeze(2).unsqueeze(3).broadcast_to([128, 2, NH, HD]),
                    op=ALU.mult,
                )
                nc.gpsimd.tensor_copy(
                    t[:].rearrange("p (c h e) -> p c h e", c=2, e=33)[:, :, :, 32],
                    keepVb[:, pg0 : pg0 + 2].unsqueeze(2).broadcast_to([128, 2, NH]),
                )
                vb.append(t)
            s_["kT"], s_["qT"], s_["vb"] = kT, qT, vb

        def mid_d(g):
            s_ = st[g]
            kT, qT = s_["kT"], s_["qT"]
            psLg = s_["psLg"]
            for s in range(S):
                c = s % 2
                for h in range(NH):
                    hb, j = h // 4, h % 4
                    col = 512 * j + (s // 2) * 32 + hb * 16
                    nc.tensor.matmul(
                        psLg[64 * c : 64 * c + 64, col : col + 16],
                        kT[hb][32 * j : 32 * j + 32, s * 64 : (s + 1) * 64],
                        qT[32 * j : 32 * j + 32, hb * 128 + s * 16 : hb * 128 + s * 16 + 16],
                        start=True, stop=True,
                        tile_position=(32 * j, 64 * c),
                    )
            ET = etp.tile([128, 544], BF16, tag="ET", name=f"ET{g}")
            psl_v = (
                psLg[:].rearrange("p (j r) -> p j r", j=4)[:, :, 0:128]
                .rearrange("p j (p2 hb q) -> p p2 hb j q", p2=4, hb=2)
            )
            et_v = ET[:, :512].rearrange(
                "p (p2 hb j q) -> p p2 hb j q", p2=4, hb=2, j=4
            )
            nc.scalar.activation(et_v, psl_v, AF.Exp, scale=SCALE)
            nc.gpsimd.memset(ET[:, 512:544], 0.0)
            s_["ET"] = ET

        def mid_e(g):
            s_ = st[g]
            ET, vb = s_["ET"], s_["vb"]
            psAt = [
                psA.tile([128, 512], F32, tag="a", name=f"psA{g}_{b}")
                for b in range(2)
            ]
            for s in range(S):
                b, strip, c = s // 4, s % 4, s % 2
                half, p2 = s // 4, (s // 2) % 2
                for h in range(NH):
                    slot = (s // 2) * 8 + h
                    nc.tensor.matmul(
                        psAt[b][32 * strip : 32 * strip + 32, 33 * h : 33 * h + 33],
                        ET[64 * c : 64 * c + 64, slot * 16 : slot * 16 + 32],
                        vb[half][64 * c : 64 * c + 64, p2 * 264 + 33 * h : p2 * 264 + 33 * h + 33],
                        start=True, stop=True,
                        tile_position=(64 * c, 32 * strip),
                    )
            s_["psAt"] = psAt

        def mid_f1(g):
            s_ = st[g]
            psAt = s_["psAt"]
            an = []
            for b in range(2):
                rec = smp.tile([128, 8], F32, tag=f"rec{b}", name=f"rec{g}_{b}")
                nc.vector.reciprocal(
                    rec[:],
                    psAt[b][:, :264].rearrange("p (h e) -> p h e", e=33)[:, :, 32],
                )
                t = anp.tile([128, 256], BF16, tag=f"an{b}", name=f"an{g}_{b}")
                nc.vector.tensor_tensor(
                    out=t[:].rearrange("p (h d) -> p h d", h=NH),
                    in0=psAt[b][:, :264].rearrange("p (h e) -> p h e", e=33)[:, :, :32],
                    in1=rec[:].unsqueeze(2).broadcast_to([128, NH, HD]),
                    op=ALU.mult,
                )
                an.append(t)
            s_["an"] = an

        def mid_f2(g):
            s_ = st[g]
            an = s_["an"]
            psT = psA.tile([128, 512], F32, tag="a", name=f"psT{g}")
            psTb = psT[:].bitcast(BF16)
            for hb in range(2):
                for b in range(2):
                    nc.tensor.transpose(
                        psTb[:, hb * 256 + b * 128 : hb * 256 + (b + 1) * 128],
                        an[b][:, hb * 128 : (hb + 1) * 128],
                        identB[:],
                    )
            aTm = atp.tile([128, 512], BF16, tag="aTm", name=f"aT{g}")
            nc.vector.tensor_copy(aTm[:], psTb[:, :512])
            aT = [aTm[:, :256], aTm[:, 256:]]
            s_["aT"] = aT

        def back_g1(g):
            s_ = st[g]
            aT = s_["aT"]
            psO = psA.tile([128, 512], F32, tag="a", name=f"psO{g}")
            for ob in range(2):
                for hb in range(2):
                    nc.tensor.matmul(
                        psO[:, ob * 128 : (ob + 1) * 128],
                        woT[hb][:, ob * 128 : (ob + 1) * 128],
                        aT[hb]
                        .rearrange("p (b st q) -> p b st q", b=2, q=32)[:, :, :, :16],
                        start=(hb == 0), stop=(hb == 1),
                    )
            ao = []
            for ob in range(2):
                t = aop.tile([128, 128], BF16, tag=f"ao{ob}", name=f"ao{g}_{ob}")
                nc.scalar.activation(
                    t[:], psO[:, ob * 128 : (ob + 1) * 128], AF.Identity,
                    bias=bosb[:, ob : ob + 1],
                )
                ao.append(t)
            s_["psO"], s_["ao"] = psO, ao

        def back_g2(g):
            s_ = st.pop(g)
            psO, ao = s_["psO"], s_["ao"]
            for ib in range(2):
                nc.tensor.matmul(
                    psO[0:M, 256:384], w2T[ib][:], ao[ib][:],
                    start=(ib == 0), stop=(ib == 1),
                )
            ot = otp.tile([M, 128], BF16, tag="ot", name=f"ot{g}")
            nc.scalar.activation(ot[:], psO[0:M, 256:384], AF.Identity, bias=b2sb[:])
            psFb = psO[:].bitcast(BF16)
            nc.tensor.transpose(psFb[:, 768:800], ot[:], identB[:M, :M])
            osb = otp.tile([128, M], F32, tag="osb", name=f"osb{g}")
            nc.scalar.activation(
                osb[:], psFb[:, 768:800], AF.Copy, scale=keepAf[:, g : g + 1]
            )
            nc.sync.dma_start(
                out=out_d.rearrange("b q m -> (b q) m")[g * 128 : (g + 1) * 128],
                in_=osb[:],
            )

        for i in range(ng + 2):
            if i < ng:
                if i + 3 < ng:
                    emit_ent_dma(i + 3)
                front_a(i)
                front_b(i)
            if 0 <= i - 1 < ng:
                mid_e(i - 1)
                mid_f1(i - 1)
            if i < ng:
                front_c(i)
            if 0 <= i - 1 < ng:
                mid_f2(i - 1)
            if i < ng:
                mid_d(i)
            if 0 <= i - 1 < ng:
                back_g1(i - 1)
            if 0 <= i - 2 < ng:
                back_g2(i - 2)

    nc.compile()
    return nc


_NC_CACHE = {}


def get_nc(bs_l=BS_L):
    if bs_l not in _NC_CACHE:
        _NC_CACHE[bs_l] = build_nc(bs_l)
    return _NC_CACHE[bs_l]


def kernel(entities, entity_mask, W1, b1, Win, Wout, bout, W2, b2):
    from concourse.bass_utils import run_bass_kernel_spmd

    entities = np.ascontiguousarray(np.asarray(entities), dtype=np.float32)
    entity_mask = np.ascontiguousarray(np.asarray(entity_mask), dtype=np.int32)
    weights = dict(
        W1=np.asarray(W1, np.float32), b1=np.asarray(b1, np.float32),
        Win=np.asarray(Win, np.float32), Wout=np.asarray(Wout, np.float32),
        bout=np.asarray(bout, np.float32), W2=np.asarray(W2, np.float32),
        b2=np.asarray(b2, np.float32),
    )
    nc = get_nc()
    in_maps = []
    for c in range(N_CORES):
        sl = slice(c * BS_L, (c + 1) * BS_L)
        in_maps.append(
            dict(entities=entities[sl], entity_mask=entity_mask[sl], **weights)
        )
    res = run_bass_kernel_spmd(nc, in_maps, core_ids=list(range(N_CORES)))
    outs = [res.results[c]["out"].reshape(BS_L, NA, M) for c in range(N_CORES)]
    return np.concatenate(outs, axis=0)


# revision 7
# speedup vs baseline: 1.0490x; 1.0205x over previous
"""Trainium2 Bass kernel for nn_AttentionHyperNet (sparse_attention).

Full-input contract: kernel(**inputs) takes FULL unsharded inputs, returns
FULL output [2048, 16, 32] f32. Shards batch across 8 cores (data parallel).

Per-core design (BS_L=256 samples, groups of S=8 samples = 512 entity tokens):
  - all GEMMs in bf16 (1 cyc/row on PE), activations bf16 end-to-end
  - entities PE-transposed to feature-major eT; fc1/K/Q feature-major
  - V token-major via x1-token-slice-stationary GEMM (2 samples per matmul);
    entity mask folded into V during PSUM evacuation (free multiply);
    a per-head "keep" column appended to V (denominator comes out of the
    attention matmul for free)
  - logits computed TRANSPOSED (lhsT = kT strips, rhs = qT, free dim = 16
    queries) -> softmax without max-subtraction (logits are small), no
    masking needed on the exp (mask lives in V/keep)
  - attn matmul: lhsT = exp'd logitsT (stationary), rhs = Vhat [64k, 33]
    -> token-major attn + denominator; normalize via reciprocal broadcast
    on the free axis; PE-transpose back to feature-major for Wout
  - agent post-mask applied once at the end (subsumes intermediate masks)
Work is spread across ACT/DVE engines (Pool takes small SBUF-only copies).
"""

import math
import numpy as np
from contextlib import ExitStack

import concourse.bass as bass
import concourse.mybir as mybir
import concourse.tile as tile
from concourse import bacc
from concourse.masks import make_identity

F32 = mybir.dt.float32
BF16 = mybir.dt.bfloat16
I32 = mybir.dt.int32
AF = mybir.ActivationFunctionType
ALU = mybir.AluOpType

BS, NE, NA, ED, H, NH, M = 2048, 64, 16, 128, 256, 8, 32
HD = H // NH  # 32
N_CORES = 8
BS_L = BS // N_CORES  # 256
S = 8                 # samples per group
NG = BS_L // S        # 32 groups
SCALE = 1.0 / math.sqrt(HD)
EPS = 1e-30


def build_nc(bs_l=BS_L):
    ng = bs_l // S
    nc = bacc.Bacc("TRN2", target_bir_lowering=False)

    ent_d = nc.dram_tensor("entities", [bs_l, NE, ED], F32, kind="ExternalInput").ap()
    em_d = nc.dram_tensor("entity_mask", [bs_l, NE], I32, kind="ExternalInput").ap()
    w1_d = nc.dram_tensor("W1", [H, ED], F32, kind="ExternalInput").ap()
    b1_d = nc.dram_tensor("b1", [H], F32, kind="ExternalInput").ap()
    win_d = nc.dram_tensor("Win", [3 * H, H], F32, kind="ExternalInput").ap()
    wout_d = nc.dram_tensor("Wout", [H, H], F32, kind="ExternalInput").ap()
    bout_d = nc.dram_tensor("bout", [H], F32, kind="ExternalInput").ap()
    w2_d = nc.dram_tensor("W2", [M, H], F32, kind="ExternalInput").ap()
    b2_d = nc.dram_tensor("b2", [M], F32, kind="ExternalInput").ap()
    out_d = nc.dram_tensor("out", [bs_l, NA, M], F32, kind="ExternalOutput").ap()

    with tile.TileContext(nc) as tc, ExitStack() as ctx:
        # ---------------- pools ----------------
        wpool = ctx.enter_context(tc.tile_pool(name="weights", bufs=1))
        pre = ctx.enter_context(tc.tile_pool(name="pre", bufs=2))
        entp = ctx.enter_context(tc.tile_pool(name="entp", bufs=5))
        eTp = ctx.enter_context(tc.tile_pool(name="eTp", bufs=3))
        x1p = ctx.enter_context(tc.tile_pool(name="x1p", bufs=3))
        ktp = ctx.enter_context(tc.tile_pool(name="ktp", bufs=3))
        qtp = ctx.enter_context(tc.tile_pool(name="qtp", bufs=3))
        vbp = ctx.enter_context(tc.tile_pool(name="vbp", bufs=3))
        etp = ctx.enter_context(tc.tile_pool(name="etp", bufs=3))
        smp = ctx.enter_context(tc.tile_pool(name="smp", bufs=3))
        anp = ctx.enter_context(tc.tile_pool(name="anp", bufs=3))
        atp = ctx.enter_context(tc.tile_pool(name="atp", bufs=3))
        aop = ctx.enter_context(tc.tile_pool(name="aop", bufs=3))
        otp = ctx.enter_context(tc.tile_pool(name="otp", bufs=3))

        psS = ctx.enter_context(tc.tile_pool(name="psS", bufs=2, space="PSUM"))
        psL = ctx.enter_context(tc.tile_pool(name="psL", bufs=1, space="PSUM"))
        psA = ctx.enter_context(tc.tile_pool(name="psA", bufs=2, space="PSUM"))

        # ---------------- preamble ----------------
        ent_tiles = {}

        def emit_ent_dma(g):
            ent = entp.tile([128, 512], F32, tag="ent", name=f"ent{g}")
            nc.sync.dma_start(
                out=ent[:].rearrange("p (tb e) -> p tb e", tb=4),
                in_=ent_d[g * S : (g + 1) * S]
                .rearrange("s n e -> (s n) e")
                .rearrange("(tb p) e -> p tb e", p=128),
            )
            ent_tiles[g] = ent

        for _g in range(3):
            emit_ent_dma(_g)

        identF = wpool.tile([128, 128], F32, tag="identF")
        make_identity(nc, identF[:])
        identB = wpool.tile([128, 128], BF16, tag="identB")
        make_identity(nc, identB[:])

        # raw weight loads (fp32, row-tiled into one sbuf tile each)
        w1raw = pre.tile([128, 256], F32, tag="w1raw", name="w1raw")
        nc.sync.dma_start(
            out=w1raw[:].rearrange("p (t e) -> p t e", t=2),
            in_=w1_d.rearrange("(t p) e -> p t e", p=128),
        )
        winraw = pre.tile([128, 1536], F32, tag="winraw", name="winraw")
        for w3 in range(3):
            nc.sync.dma_start(
                out=winraw[:, w3 * 512 : (w3 + 1) * 512]
                .rearrange("p (t h) -> p t h", t=2),
                in_=win_d.rearrange("(t p) h -> p t h", p=128)[:, 2 * w3 : 2 * w3 + 2],
            )
        woraw = pre.tile([128, 512], F32, tag="woraw", name="woraw")
        nc.sync.dma_start(
            out=woraw[:].rearrange("p (t h) -> p t h", t=2),
            in_=wout_d.rearrange("(t p) h -> p t h", p=128),
        )
        w2raw = pre.tile([32, 256], F32, tag="w2raw", name="w2raw")
        nc.sync.dma_start(out=w2raw[:], in_=w2_d)

        # W1T [ED, H] bf16
        w1T = wpool.tile([128, 256], BF16, tag="w1T")
        ps = psS.tile([128, 512], F32, tag="ps", name="psw1")
        for t in range(2):
            nc.tensor.transpose(
                ps[:, t * 128 : (t + 1) * 128], w1raw[:, t * 128 : (t + 1) * 128],
                identF[:],
            )
        nc.scalar.activation(w1T[:], ps[:, :256], AF.Copy)

        def make_wT(raw_ap, t0, tag):
            """rows [t0*128:(t0+2)*128] of a [*,256] weight -> transposed bf16
            tiles XT[ib] [128 (in-half), 256 (out)]."""
            tiles = []
            for ib in range(2):
                t = wpool.tile([128, 256], BF16, tag=f"{tag}{ib}", name=f"{tag}{ib}")
                ps = psS.tile([128, 512], F32, tag="ps", name=f"psw{tag}{ib}")
                for ta in range(2):
                    nc.tensor.transpose(
                        ps[:, ta * 128 : (ta + 1) * 128],
                        raw_ap[:, (t0 + ta) * 256 + ib * 128 : (t0 + ta) * 256 + (ib + 1) * 128],
                        identF[:],
                    )
                nc.scalar.activation(t[:], ps[:, :256], AF.Copy)
                tiles.append(t)
            return tiles

        wqT = make_wT(winraw, 0, "wqT")
        wkT = make_wT(winraw, 2, "wkT")
        wvT = make_wT(winraw, 4, "wvT")
        woT = make_wT(woraw, 0, "woT")

        w2T = []
        for ib in range(2):
            t = wpool.tile([128, 32], BF16, tag=f"w2T{ib}", name=f"w2T{ib}")
            ps = psS.tile([128, 512], F32, tag="ps", name=f"psw2{ib}")
            nc.tensor.transpose(
                ps[:, :32], w2raw[:, ib * 128 : (ib + 1) * 128], identF[:32, :32]
            )
            nc.vector.tensor_copy(t[:], ps[:, :32])
            w2T.append(t)

        b1sb = wpool.tile([128, 2], F32, tag="b1sb")
        nc.sync.dma_start(out=b1sb[:], in_=b1_d.rearrange("(t p) -> p t", p=128))
        bosb = wpool.tile([128, 2], F32, tag="bosb")
        nc.sync.dma_start(out=bosb[:], in_=bout_d.rearrange("(t p) -> p t", p=128))
        b2sb = wpool.tile([32, 1], F32, tag="b2sb")
        nc.sync.dma_start(out=b2sb[:], in_=b2_d.rearrange("(p o) -> p o", o=1))
        bosbB = wpool.tile([128, 2], BF16, tag="bosbB")
        nc.gpsimd.tensor_copy(bosbB[:], bosb[:])
        psb2 = psS.tile([128, 512], F32, tag="ps", name="psb2")
        for ib in range(2):
            nc.tensor.matmul(
                psb2[0:M, 0:1], w2T[ib][:], bosbB[:, ib : ib + 1],
                start=(ib == 0), stop=(ib == 1),
            )
        b2p = wpool.tile([M, 1], F32, tag="b2p")
        nc.scalar.activation(b2p[:], psb2[0:M, 0:1], AF.Identity, bias=b2sb[:])

        # keep masks: keepV [128=(sp,k), bs_l//2 (sample-pair)] = 1 - em
        emV = pre.tile([128, bs_l // 2], I32, tag="emV", name="emV")
        nc.sync.dma_start(
            out=emV[:], in_=em_d.rearrange("(p two) n -> (two n) p", two=2)
        )
        keepVf = wpool.tile([128, bs_l // 2], F32, tag="keepVf")
        nc.vector.tensor_scalar(
            out=keepVf[:], in0=emV[:], scalar1=-1.0, scalar2=1.0,
            op0=ALU.mult, op1=ALU.add,
        )
        keepVb = wpool.tile([128, bs_l // 2], BF16, tag="keepVb")
        nc.gpsimd.tensor_scalar_add(keepVb[:], keepVf[:], 1e-30)

        # keepA [128=(s,q), ng] for the final agent post-mask
        emA = pre.tile([128, ng], I32, tag="emA", name="emA")
        for s8 in range(S):
            nc.sync.dma_start(
                out=emA[s8 * NA : (s8 + 1) * NA, :],
                in_=em_d[:, 0:NA].rearrange("(g s) q -> s q g", s=S)[s8],
            )
        keepAf = wpool.tile([128, ng], F32, tag="keepAf")
        nc.vector.tensor_scalar(
            out=keepAf[:], in0=emA[:], scalar1=-1.0, scalar2=1.0,
            op0=ALU.mult, op1=ALU.add,
        )

        # ---------------- main loop ----------------

        st = {}  # per-group state

        def front_a(g):
            ent = ent_tiles.pop(g)
            entb = entp.tile([128, 512], BF16, tag="entb", name=f"entb{g}")
            nc.gpsimd.tensor_copy(entb[:], ent[:])
            psLg = psL.tile([128, 2048], F32, tag="l", name=f"psL{g}")
            psEb = psLg[:].bitcast(BF16)[:, 256:768]
            for tb in range(4):
                nc.tensor.transpose(
                    psEb[:, tb * 128 : (tb + 1) * 128],
                    entb[:, tb * 128 : (tb + 1) * 128],
                    identB[:],
                )
            eT = eTp.tile([128, 512], BF16, tag="eT", name=f"eT{g}")
            nc.vector.tensor_copy(eT[:], psEb[:])
            st[g] = dict(eT=eT, psLg=psLg)

        def front_b(g):
            s_ = st[g]
            x1 = []
            for hb in range(2):
                psX = psS.tile([128, 512], F32, tag="ps", name=f"psX{g}_{hb}")
                nc.tensor.matmul(
                    psX[:], w1T[:, hb * 128 : (hb + 1) * 128], s_["eT"][:],
                    start=True, stop=True,
                )
                t = x1p.tile([128, 512], BF16, tag=f"x1{hb}", name=f"x1_{g}_{hb}")
                nc.scalar.activation(t[:], psX[:], AF.Relu, bias=b1sb[:, hb : hb + 1])
                x1.append(t)
            s_["x1"] = x1

        def front_c(g):
            s_ = st[g]
            x1 = s_["x1"]
            kTm = ktp.tile([128, 1024], BF16, tag="kTm", name=f"kT{g}")
            psKa = psS.tile([128, 512], F32, tag="ps", name=f"psK{g}_0")
            psKb = psS.tile([128, 512], F32, tag="ps", name=f"psK{g}_1")
            for hb, psK in enumerate((psKa, psKb)):
                for ib in range(2):
                    nc.tensor.matmul(
                        psK[:], wkT[ib][:, hb * 128 : (hb + 1) * 128], x1[ib][:],
                        start=(ib == 0), stop=(ib == 1),
                    )
                if hb == 0:
                    nc.scalar.activation(kTm[:, :512], psK[:], AF.Copy)
                else:
                    nc.vector.tensor_copy(kTm[:, 512:], psK[:])
            kT = [kTm[:, :512], kTm[:, 512:]]
            psQ = s_["psLg"][:, 640:896]
            for hb in range(2):
                for ib in range(2):
                    nc.tensor.matmul(
                        psQ[:, hb * 128 : (hb + 1) * 128],
                        wqT[ib][:, hb * 128 : (hb + 1) * 128],
                        x1[ib][:].rearrange("p (s n) -> p s n", s=S)[:, :, :NA],
                        start=(ib == 0), stop=(ib == 1),
                    )
            qT = qtp.tile([128, 256], BF16, tag="qT", name=f"qT{g}")
            nc.scalar.activation(qT[:], psQ[:, :256], AF.Copy)
            vb = []
            for half in range(2):
                psV = psS.tile([128, 512], F32, tag="ps", name=f"psV{g}_{half}")
                for p2 in range(2):
                    p = half * 2 + p2
                    for ib in range(2):
                        nc.tensor.matmul(
                            psV[:, p2 * 256 : (p2 + 1) * 256],
                            x1[ib][:, p * 128 : (p + 1) * 128],
                            wvT[ib][:],
                            start=(ib == 0), stop=(ib == 1),
                        )
                t = vbp.tile([128, 2 * 264], BF16, tag=f"vb{half}", name=f"vb{g}_{half}")
                pg0 = g * 4 + half * 2
                nc.vector.tensor_tensor(
                    out=t[:].rearrange("p (c h e) -> p c h e", c=2, e=33)[:, :, :, :32],
                    in0=psV[:].rearrange("p (c h d) -> p c h d", c=2, h=NH),
                    in1=keepVf[:, pg0 : pg0 + 2]
                    .unsqueeze(2).unsqueeze(3).broadcast_to([128, 2, NH, HD]),
                    op=ALU.mult,
                )
                nc.gpsimd.tensor_copy(
                    t[:].rearrange("p (c h e) -> p c h e", c=2, e=33)[:, :, :, 32],
                    keepVb[:, pg0 : pg0 + 2].unsqueeze(2).broadcast_to([128, 2, NH]),
                )
                vb.append(t)
            s_["kT"], s_["qT"], s_["vb"] = kT, qT, vb

        def mid_d(g):
            s_ = st[g]
            kT, qT = s_["kT"], s_["qT"]
            psLg = s_["psLg"]
            for s in range(S):
                c = s % 2
                for h in range(NH):
                    hb, j = h // 4, h % 4
                    col = 512 * j + (s // 2) * 32 + hb * 16
                    nc.tensor.matmul(
                        psLg[64 * c : 64 * c + 64, col : col + 16],
                        kT[hb][32 * j : 32 * j + 32, s * 64 : (s + 1) * 64],
                        qT[32 * j : 32 * j + 32, hb * 128 + s * 16 : hb * 128 + s * 16 + 16],
                        start=True, stop=True,
                        tile_position=(32 * j, 64 * c),
                    )
            ET = etp.tile([128, 544], BF16, tag="ET", name=f"ET{g}")
            psl_v = (
                psLg[:].rearrange("p (j r) -> p j r", j=4)[:, :, 0:128]
                .rearrange("p j (p2 hb q) -> p p2 hb j q", p2=4, hb=2)
            )
            et_v = ET[:, :512].rearrange(
                "p (p2 hb j q) -> p p2 hb j q", p2=4, hb=2, j=4
            )
            nc.scalar.activation(et_v, psl_v, AF.Exp, scale=SCALE)
            nc.gpsimd.memset(ET[:, 512:544], 0.0)
            s_["ET"] = ET

        def mid_e(g):
            s_ = st[g]
            ET, vb = s_["ET"], s_["vb"]
            psAt = [
                psA.tile([128, 512], F32, tag="a", name=f"psA{g}_{b}")
                for b in range(2)
            ]
            for s in range(S):
                b, strip, c = s // 4, s % 4, s % 2
                half, p2 = s // 4, (s // 2) % 2
                for h in range(NH):
                    slot = (s // 2) * 8 + h
                    nc.tensor.matmul(
                        psAt[b][32 * strip : 32 * strip + 32, 33 * h : 33 * h + 33],
                        ET[64 * c : 64 * c + 64, slot * 16 : slot * 16 + 32],
                        vb[half][64 * c : 64 * c + 64, p2 * 264 + 33 * h : p2 * 264 + 33 * h + 33],
                        start=True, stop=True,
                        tile_position=(64 * c, 32 * strip),
                    )
            s_["psAt"] = psAt

        def mid_f1(g):
            s_ = st[g]
            psAt = s_["psAt"]
            an = []
            for b in range(2):
                rec = smp.tile([128, 8], F32, tag=f"rec{b}", name=f"rec{g}_{b}")
                nc.vector.reciprocal(
                    rec[:],
                    psAt[b][:, :264].rearrange("p (h e) -> p h e", e=33)[:, :, 32],
                )
                t = anp.tile([128, 256], BF16, tag=f"an{b}", name=f"an{g}_{b}")
                nc.vector.tensor_tensor(
                    out=t[:].rearrange("p (h d) -> p h d", h=NH),
                    in0=psAt[b][:, :264].rearrange("p (h e) -> p h e", e=33)[:, :, :32],
                    in1=rec[:].unsqueeze(2).broadcast_to([128, NH, HD]),
                    op=ALU.mult,
                )
                an.append(t)
            s_["an"] = an

        def mid_f2(g):
            s_ = st[g]
            an = s_["an"]
            psT = psA.tile([128, 512], F32, tag="a", name=f"psT{g}")
            psTb = psT[:].bitcast(BF16)
            for hb in range(2):
                for b in range(2):
                    nc.tensor.transpose(
                        psTb[:, hb * 256 + b * 128 : hb * 256 + (b + 1) * 128],
                        an[b][:, hb * 128 : (hb + 1) * 128],
                        identB[:],
                    )
            aTm = atp.tile([128, 512], BF16, tag="aTm", name=f"aT{g}")
            nc.vector.tensor_copy(aTm[:], psTb[:, :512])
            aT = [aTm[:, :256], aTm[:, 256:]]
            s_["aT"] = aT

        def back_g1(g):
            s_ = st[g]
            aT = s_["aT"]
            psO = psA.tile([128, 512], F32, tag="a", name=f"psO{g}")
            for ob in range(2):
                for hb in range(2):
                    nc.tensor.matmul(
                        psO[:, ob * 128 : (ob + 1) * 128],
                        woT[hb][:, ob * 128 : (ob + 1) * 128],
                        aT[hb]
                        .rearrange("p (b st q) -> p b st q", b=2, q=32)[:, :, :, :16],
                        start=(hb == 0), stop=(hb == 1),
                    )
            aom = aop.tile([128, 256], BF16, tag="aom", name=f"ao{g}")
            nc.scalar.activation(aom[:], psO[:, 0:256], AF.Copy)
            s_["psO"], s_["ao"] = psO, aom

        def back_g2(g):
            s_ = st.pop(g)
            psO, ao = s_["psO"], s_["ao"]
            for ib in range(2):
                nc.tensor.matmul(
                    psO[0:M, 256:384], w2T[ib][:], ao[:, ib * 128 : (ib + 1) * 128],
                    start=(ib == 0), stop=(ib == 1),
                )
            ot = otp.tile([M, 128], BF16, tag="ot", name=f"ot{g}")
            nc.scalar.activation(ot[:], psO[0:M, 256:384], AF.Identity, bias=b2p[:])
            psFb = psO[:].bitcast(BF16)
            nc.tensor.transpose(psFb[:, 768:800], ot[:], identB[:M, :M])
            osb = otp.tile([128, M], F32, tag="osb", name=f"osb{g}")
            nc.scalar.activation(
                osb[:], psFb[:, 768:800], AF.Copy, scale=keepAf[:, g : g + 1]
            )
            nc.sync.dma_start(
                out=out_d.rearrange("b q m -> (b q) m")[g * 128 : (g + 1) * 128],
                in_=osb[:],
            )

        for i in range(ng + 2):
            if i < ng:
                if i + 3 < ng:
                    emit_ent_dma(i + 3)
                front_a(i)
                front_b(i)
            if 0 <= i - 1 < ng:
                mid_e(i - 1)
                mid_f1(i - 1)
            if i < ng:
                front_c(i)
            if 0 <= i - 1 < ng:
                mid_f2(i - 1)
            if i < ng:
                mid_d(i)
            if 0 <= i - 1 < ng:
                back_g1(i - 1)
            if 0 <= i - 2 < ng:
                back_g2(i - 2)

    nc.compile()
    return nc


_NC_CACHE = {}


def get_nc(bs_l=BS_L):
    if bs_l not in _NC_CACHE:
        _NC_CACHE[bs_l] = build_nc(bs_l)
    return _NC_CACHE[bs_l]


def kernel(entities, entity_mask, W1, b1, Win, Wout, bout, W2, b2):
    from concourse.bass_utils import run_bass_kernel_spmd

    entities = np.ascontiguousarray(np.asarray(entities), dtype=np.float32)
    entity_mask = np.ascontiguousarray(np.asarray(entity_mask), dtype=np.int32)
    weights = dict(
        W1=np.asarray(W1, np.float32), b1=np.asarray(b1, np.float32),
        Win=np.asarray(Win, np.float32), Wout=np.asarray(Wout, np.float32),
        bout=np.asarray(bout, np.float32), W2=np.asarray(W2, np.float32),
        b2=np.asarray(b2, np.float32),
    )
    nc = get_nc()
    in_maps = []
    for c in range(N_CORES):
        sl = slice(c * BS_L, (c + 1) * BS_L)
        in_maps.append(
            dict(entities=entities[sl], entity_mask=entity_mask[sl], **weights)
        )
    res = run_bass_kernel_spmd(nc, in_maps, core_ids=list(range(N_CORES)))
    outs = [res.results[c]["out"].reshape(BS_L, NA, M) for c in range(N_CORES)]
    return np.concatenate(outs, axis=0)
